# revision 36
# baseline (speedup 1.0000x reference)
"""8-core Trainium2 Bass kernel for nn_Attention_86079734546756.

Sharding: B=4 batches x 2 head-groups (8 heads each) -> 8 cores.
Per core (batch b, head-group g):
  - fp16 projections (Q/K transposed-out, V natural with an appended ones
    column for softmax denominators)
  - scores S^T[k,q] = K_h Q_h^T via fp16 matmuls, two heads packed into the
    128-row PE array (dh=64 contraction each, base_partition 0/64)
  - exp on ScalarE (PSUM->SBUF, bf16 out), mask multiply on VectorE
  - P^T V via bf16 matmuls with M=65 (65th row = ones -> denominators)
  - both heads' denominators staged to SBUF rows 0/64 so ONE batched
    DVE reciprocal (its ~3.3us ucode cost is free-size-bound) serves the
    pair; normalize via reciprocal broadcast across partitions with two
    K=1 bf16 matmuls (hi/lo split for ~fp32 precision)
  - output projection (bf16) -> partial y summed on host (+ wo_b)
Attention emission is software-pipelined (PV of item i-1 after scores of
item i) and a post-pass splits multi-wait instructions because this
walrus build encodes one sync wait per instruction.
No max-subtraction in softmax: |alpha| <~ 60 so exp stays in fp32 range,
and masked lanes are zeroed exactly by multiplying with (1-mask) after exp.
"""

import os
import numpy as np
import ml_dtypes

B, S, D, H = 4, 2048, 1024, 16
DH = D // H          # 64
P = 128
HPC = 8              # heads per core
OC = 512             # output features per core (head-group width)
NKT = S // P         # 16 k-tiles
NQC = S // 512       # 4 q-chunks
NOT = OC // P        # 4 o-tiles
NDT = D // P         # 8 d-tiles
N_CORES = 8

LAST_EXEC_NS = None
LAST_RESULTS = None

_BF16 = ml_dtypes.bfloat16


def _split_multi_waits(nc, mybir, max_waits: int = 1):
    """The walrus build in this container encodes at most one sync wait per
    ISA instruction and refuses to split. Move extra waits onto standalone
    EventSemaphore instructions inserted just before, on the same engine —
    the engine executes them in stream order, so semantics are unchanged
    (DMA triggers are simply enqueued after the waits pass)."""
    ctr = 0
    for fn in nc.m.functions:
        for blk in fn.blocks:
            insts = blk.instructions
            if not any(
                inst.sync_info is not None
                and inst.sync_info.on_wait
                and len(inst.sync_info.on_wait) > max_waits
                for inst in insts
            ):
                continue
            out = []
            for inst in insts:
                si = inst.sync_info
                waits = list(si.on_wait) if si is not None and si.on_wait else []
                if len(waits) > max_waits:
                    extra, keep = waits[:-max_waits], waits[-max_waits:]
                    for w in extra:
                        ev = mybir.InstEventSemaphore(
                            name=f"evsplit-{ctr}",
                            engine=inst.engine,
                            ins=[],
                            outs=[],
                            sync_info=mybir.SyncInfo(on_wait=[w], on_update=[]),
                        )
                        ctr += 1
                        out.append(ev)
                    si.on_wait = keep
                out.append(inst)
            blk.instructions = out
    return ctr


def _build_program(with_qkv_bias: bool):
    from contextlib import ExitStack
    import concourse.bass as bass
    import concourse.mybir as mybir
    import concourse.tile as tile

    dt = mybir.dt
    AF = mybir.ActivationFunctionType
    ALU = mybir.AluOpType

    nc = bass.Bass(trn_type="TRN2")

    xq = nc.declare_dram_parameter("xq_t", [D, S], dt.float16, isOutput=False)
    xk = nc.declare_dram_parameter("xk_t", [D, S], dt.float16, isOutput=False)
    xv = nc.declare_dram_parameter("xv_t", [D, S], dt.float16, isOutput=False)
    invm = nc.declare_dram_parameter("invm_t", [S, S], dt.bfloat16, isOutput=False)
    wq = nc.declare_dram_parameter("wq_t", [D, OC], dt.float16, isOutput=False)
    wk = nc.declare_dram_parameter("wk_t", [D, OC], dt.float16, isOutput=False)
    wv = nc.declare_dram_parameter("wv_t", [D, OC], dt.float16, isOutput=False)
    wo = nc.declare_dram_parameter("wo_t", [OC, D], dt.bfloat16, isOutput=False)
    if with_qkv_bias:
        bq = nc.declare_dram_parameter("bq", [OC], dt.float32, isOutput=False)
        bk = nc.declare_dram_parameter("bk", [OC], dt.float32, isOutput=False)
        bv = nc.declare_dram_parameter("bv_bcast", [P, OC], dt.float32, isOutput=False)
    y = nc.declare_dram_parameter("y_part", [S, D], dt.float32, isOutput=True)

    with tile.TileContext(nc) as tc, ExitStack() as ctx:
        persist = ctx.enter_context(tc.tile_pool(name="persist", bufs=1))
        wpool = ctx.enter_context(tc.tile_pool(name="wpool", bufs=2))
        xpool = ctx.enter_context(tc.tile_pool(name="xpool", bufs=4))
        invp = ctx.enter_context(tc.tile_pool(name="invp", bufs=2))
        ptp = ctx.enter_context(tc.tile_pool(name="ptp", bufs=2))
        yp = ctx.enter_context(tc.tile_pool(name="yp", bufs=3))
        smallp = ctx.enter_context(tc.tile_pool(name="smallp", bufs=1))
        scp = ctx.enter_context(tc.tile_pool(name="scp", bufs=2, space="PSUM"))
        mmp = ctx.enter_context(tc.tile_pool(name="mmp", bufs=3, space="PSUM"))
        bcp = ctx.enter_context(tc.tile_pool(name="bcp", bufs=1, space="PSUM"))

        QHT = persist.tile([P, NOT, S], dt.float16)          # [o%128, ot, s]
        KHT = persist.tile([P, NOT, S], dt.float16)
        VSB = persist.tile([P, NKT, HPC, DH + 1], dt.bfloat16)  # [k%128, kt, h, dh|1]
        OT = persist.tile([P, NOT, S], dt.bfloat16)          # [c%128, ct, s]
        WO = persist.tile([P, NOT, D], dt.bfloat16)          # [c%128, ct, o]
        ones = persist.tile([DH + 1, DH], dt.bfloat16)
        nc.vector.memset(ones[:], 1.0)
        nc.vector.memset(VSB[:, :, :, DH : DH + 1], 1.0)
        # Staging for pair-batched softmax denominators: rows 0 and 64 hold
        # the two heads' denominators (both legal PE base partitions); rows
        # 1-63 stay at 1.0 so the batched reciprocal never sees garbage.
        stages = [
            persist.tile([DH + 1, 512], dt.float32, name=f"stage{i}") for i in range(2)
        ]
        for t in stages:
            nc.vector.memset(t[:], 1.0)
        rcp = persist.tile([DH + 1, 512], dt.float32, name="rcp")

        if with_qkv_bias:
            bq_sb = persist.tile([P, NOT], dt.float32)
            nc.sync.dma_start(bq_sb[:], bq.rearrange("(ot p) -> p ot", p=P))
            bk_sb = persist.tile([P, NOT], dt.float32)
            nc.sync.dma_start(bk_sb[:], bk.rearrange("(ot p) -> p ot", p=P))
            bv_sb = persist.tile([P, OC], dt.float32)
            nc.sync.dma_start(bv_sb[:], bv[:])

        # ---------------- projections -----------------
        # Q/K interleaved per o-tile pair so attention for the first head
        # pairs can start while the rest of the projections still run.
        xqr = xq.rearrange("(dt p) s -> dt p s", p=P)
        xkr = xk.rearrange("(dt p) s -> dt p s", p=P)
        wq_sb = wpool.tile([P, NDT, OC], dt.float16, tag="w", name="wq_sb")
        nc.sync.dma_start(wq_sb[:], wq.rearrange("(dt p) o -> p dt o", p=P))
        wk_sb = wpool.tile([P, NDT, OC], dt.float16, tag="w", name="wk_sb")
        nc.sync.dma_start(wk_sb[:], wk.rearrange("(dt p) o -> p dt o", p=P))

        def proj_full(xr, wsb, dst, bias_sb):
            for sc in range(NQC):
                pss = [
                    scp.tile([P, 2, 512], dt.float32, tag="sc", name=f"pj_{i}")
                    for i in range(2)
                ]
                for dti in range(NDT):
                    xt = xpool.tile([P, 512], dt.float16, tag="x", name="xt")
                    nc.sync.dma_start(xt[:], xr[dti, :, sc * 512 : (sc + 1) * 512])
                    for ot in range(NOT):
                        nc.tensor.matmul(
                            pss[ot // 2][:, ot % 2, :],
                            lhsT=wsb[:, dti, ot * P : (ot + 1) * P],
                            rhs=xt[:],
                            start=(dti == 0),
                            stop=(dti == NDT - 1),
                        )
                for ot in range(NOT):
                    src = pss[ot // 2][:, ot % 2, :]
                    dstap = dst[:, ot, sc * 512 : (sc + 1) * 512]
                    if bias_sb is not None:
                        nc.scalar.activation(
                            dstap, src, AF.Identity, bias=bias_sb[:, ot : ot + 1]
                        )
                    else:
                        nc.scalar.activation(dstap, src, AF.Copy)

        proj_full(xkr, wk_sb, KHT, bk_sb if with_qkv_bias else None)
        proj_full(xqr, wq_sb, QHT, bq_sb if with_qkv_bias else None)

        # V: natural layout [s, o] scattered into VSB[k%128, kt, h, 0:64].
        # Uses the mmp psum tag so attention score tiles don't queue behind it.
        xvr = xv.rearrange("(dt p) s -> dt p s", p=P)
        wvsb = wpool.tile([P, NDT, OC], dt.float16, tag="w", name="wvsb")
        nc.sync.dma_start(wvsb[:], wv.rearrange("(dt p) o -> p dt o", p=P))
        for sc in range(NQC):
            for vh in range(2):
                psa = mmp.tile([P, 512], dt.float32, tag="mm", name=f"pva{sc}_{vh}")
                psb = mmp.tile([P, 512], dt.float32, tag="mm", name=f"pvb{sc}_{vh}")
                for dti in range(NDT):
                    xt = xpool.tile([P, 512], dt.float16, tag="x", name="xtv")
                    nc.sync.dma_start(
                        xt[:], xvr[dti, :, sc * 512 : (sc + 1) * 512]
                    )
                    for i, psx in enumerate((psa, psb)):
                        sti = vh * 2 + i
                        nc.tensor.matmul(
                            psx[:],
                            lhsT=xt[:, sti * P : (sti + 1) * P],
                            rhs=wvsb[:, dti, :],
                            start=(dti == 0),
                            stop=(dti == NDT - 1),
                        )
                for i, psx in enumerate((psa, psb)):
                    st = sc * 4 + vh * 2 + i
                    src = psx[:].rearrange("p (h d) -> p h d", d=DH)
                    dstap = VSB[:, st, :, 0:DH]
                    if with_qkv_bias:
                        nc.vector.tensor_tensor(
                            dstap,
                            src,
                            bv_sb[:].rearrange("p (h d) -> p h d", d=DH),
                            ALU.add,
                        )
                    else:
                        nc.vector.tensor_copy(dstap, src)

        nc.sync.dma_start(WO[:], wo.rearrange("(ct p) o -> p ct o", p=P))

        # ---------------- attention -----------------
        # Software-pipelined emission: PV/normalize for item i-1 are emitted
        # after scores/exp/mask for item i, so the scheduler keeps feeding
        # ScalarE fresh score tiles at iteration boundaries.
        imr = invm.rearrange("(kt p) q -> p kt q", p=P)
        items = [(qc, hp) for qc in range(NQC) for hp in range(NOT)]
        imqs = {}
        pts = {}

        def load_imq(qc):
            if qc in imqs or qc >= NQC:
                return
            qsl = slice(qc * 512, (qc + 1) * 512)
            imq = invp.tile([P, NKT, 512], dt.bfloat16, tag="im", name="imq")
            for k4 in range(4):
                nc.sync.dma_start(
                    imq[:, k4 * 4 : (k4 + 1) * 4, :],
                    imr[:, k4 * 4 : (k4 + 1) * 4, qsl],
                )
            imqs[qc] = imq

        def emit_scores(qc, hp):
            qsl = slice(qc * 512, (qc + 1) * 512)
            load_imq(qc)
            if hp == NOT - 2:
                load_imq(qc + 1)
            imq = imqs[qc]
            PT = ptp.tile([P, NKT, 2, 512], dt.bfloat16, tag="pt", name="PT")
            pts[(qc, hp)] = PT
            for kt in range(NKT):
                ps = scp.tile([P, 2, 512], dt.float32, tag="sc", name="sc")
                ksl = slice(kt * P, (kt + 1) * P)
                nc.tensor.matmul(
                    ps[:, 0, :],
                    lhsT=KHT[0:DH, hp, ksl],
                    rhs=QHT[0:DH, hp, qsl],
                    start=True,
                    stop=True,
                )
                nc.tensor.matmul(
                    ps[:, 1, :],
                    lhsT=KHT[DH:P, hp, ksl],
                    rhs=QHT[DH:P, hp, qsl],
                    start=True,
                    stop=True,
                )
                nc.scalar.activation(PT[:, kt, :, :], ps[:], AF.Exp)
            for j in range(2):
                for q4 in range(4):
                    nc.vector.tensor_tensor(
                        PT[:, q4 * 4 : (q4 + 1) * 4, j, :],
                        PT[:, q4 * 4 : (q4 + 1) * 4, j, :],
                        imq[:, q4 * 4 : (q4 + 1) * 4, :],
                        ALU.mult,
                    )

        stage_idx = [0]

        def emit_pv(qc, hp):
            qsl = slice(qc * 512, (qc + 1) * 512)
            PT = pts.pop((qc, hp))
            si = stage_idx[0]
            stage_idx[0] ^= 1
            stage = stages[si]
            pvs = []
            for j in range(2):
                h = hp * 2 + j
                pv = mmp.tile([P, 512], dt.float32, tag="mm", name="pv")
                for kt in range(NKT):
                    nc.tensor.matmul(
                        pv[0 : DH + 1, :],
                        lhsT=VSB[:, kt, h, :],
                        rhs=PT[:, kt, j, :],
                        start=(kt == 0),
                        stop=(kt == NKT - 1),
                    )
                if j == 0:
                    # Hop through SBUF; the DMA moves the denominator from
                    # partition 64 to row 0 of the staging tile.
                    dtmp = smallp.tile(
                        [DH + 1, 512], dt.float32, tag="dtmp", name="dtmp"
                    )
                    nc.vector.tensor_copy(dtmp[DH : DH + 1, :], pv[DH : DH + 1, :])
                    nc.sync.dma_start(stage[0:1, :], dtmp[DH : DH + 1, :])
                else:
                    nc.vector.tensor_copy(stage[DH : DH + 1, :], pv[DH : DH + 1, :])
                pvs.append(pv)
            # One batched reciprocal serves both heads (rows 0 and 64; rows
            # 1-63 run on the stage's constant 1.0 filler so every lane
            # stays finite); the ~3.3us DVE ucode cost is free-size-bound,
            # not row-bound.
            nc.vector.reciprocal(rcp[:], stage[:])
            # hi/lo bf16 split: bf16 covers the reciprocals' exponent
            # range (down to ~1e-27); accumulating hi+lo in PSUM
            # recovers ~bf16^2 relative precision.
            rc_hi = smallp.tile([DH + 1, 512], dt.bfloat16, tag="rch", name="rc_hi")
            nc.vector.tensor_copy(rc_hi[:], rcp[:])
            nc.vector.tensor_tensor(rcp[:], rcp[:], rc_hi[:], ALU.subtract)
            rc_lo = smallp.tile([DH + 1, 512], dt.bfloat16, tag="rcl", name="rc_lo")
            nc.vector.tensor_copy(rc_lo[:], rcp[:])
            for j in range(2):
                b = j * DH
                bc = bcp.tile([DH, 512], dt.float32, tag="bc", name="bc")
                nc.tensor.matmul(
                    bc[:],
                    lhsT=ones[b : b + 1, :],
                    rhs=rc_hi[b : b + 1, :],
                    start=True,
                    stop=False,
                )
                nc.tensor.matmul(
                    bc[:],
                    lhsT=ones[b : b + 1, :],
                    rhs=rc_lo[b : b + 1, :],
                    start=False,
                    stop=True,
                )
                bcs = smallp.tile([DH, 512], dt.float32, tag="bcs", name="bcs")
                nc.vector.tensor_copy(bcs[:], bc[:])
                nc.vector.tensor_tensor(
                    OT[j * DH : (j + 1) * DH, hp, qsl],
                    pvs[j][0:DH, :],
                    bcs[:],
                    ALU.mult,
                )

        def emit_outproj(qc):
            yr = y.rearrange("(st p) o -> st p o", p=P)
            for sti in range(4):
                st = qc * 4 + sti
                ssl = slice(st * P, (st + 1) * P)
                for oc2 in range(2):
                    osl = slice(oc2 * 512, (oc2 + 1) * 512)
                    op = mmp.tile([P, 512], dt.float32, tag="mm", name="op")
                    for ct in range(NOT):
                        nc.tensor.matmul(
                            op[:],
                            lhsT=OT[:, ct, ssl],
                            rhs=WO[:, ct, osl],
                            start=(ct == 0),
                            stop=(ct == NOT - 1),
                        )
                    yt = yp.tile([P, 512], dt.float32, tag="y", name="yt")
                    nc.vector.tensor_copy(yt[:], op[:])
                    nc.sync.dma_start(yr[st, :, osl], yt[:])

        for idx in range(len(items) + 1):
            if idx < len(items):
                emit_scores(*items[idx])
            if idx > 0:
                pqc, php = items[idx - 1]
                emit_pv(pqc, php)
                if php == NOT - 1:
                    emit_outproj(pqc)

    n_split = _split_multi_waits(nc, mybir)
    return nc


def kernel(q, k, v, mask, wq_w, wq_b, wk_w, wk_b, wv_w, wv_b, wo_w, wo_b):
    global LAST_EXEC_NS, LAST_RESULTS
    from concourse.bass_utils import run_bass_kernel_spmd

    q = np.asarray(q, np.float32)
    k = np.asarray(k, np.float32)
    v = np.asarray(v, np.float32)
    mask = np.asarray(mask)
    wq_w = np.asarray(wq_w, np.float32)
    wk_w = np.asarray(wk_w, np.float32)
    wv_w = np.asarray(wv_w, np.float32)
    wo_w = np.asarray(wo_w, np.float32)
    wq_b = np.asarray(wq_b, np.float32)
    wk_b = np.asarray(wk_b, np.float32)
    wv_b = np.asarray(wv_b, np.float32)
    wo_b = np.asarray(wo_b, np.float32)

    nc, in_maps, with_qkv_bias = prepare(
        q, k, v, mask, wq_w, wq_b, wk_w, wk_b, wv_w, wv_b, wo_w, wo_b
    )

    trace = bool(int(os.environ.get("BASS_ATTN_TRACE", "0")))
    trace_cores = None
    tc_env = os.environ.get("BASS_ATTN_TRACE_CORES", "")
    if tc_env:
        trace_cores = [int(c) for c in tc_env.split(",")]
    res = run_bass_kernel_spmd(
        nc,
        in_maps,
        core_ids=list(range(N_CORES)),
        trace=trace,
        trace_cores=trace_cores,
    )
    LAST_EXEC_NS = res.exec_time_ns
    LAST_RESULTS = res

    out = np.zeros((B, S, D), np.float32)
    for b in range(B):
        out[b] = (
            res.results[2 * b]["y_part"]
            + res.results[2 * b + 1]["y_part"]
            + wo_b[None, :]
        )
    return out


_NC_CACHE = {}


def prepare(q, k, v, mask, wq_w, wq_b, wk_w, wk_b, wv_w, wv_b, wo_w, wo_b):
    with_qkv_bias = bool(
        np.asarray(wq_b).any() or np.asarray(wk_b).any() or np.asarray(wv_b).any()
    )
    if with_qkv_bias not in _NC_CACHE:
        _NC_CACHE[with_qkv_bias] = _build_program(with_qkv_bias)
    nc = _NC_CACHE[with_qkv_bias]

    in_maps = []
    xts = {}
    invs = {}
    for b in range(B):
        # Cast before transposing: halves the bytes the transpose moves.
        xts[b] = (
            np.ascontiguousarray(q[b].astype(np.float16).T),
            np.ascontiguousarray(k[b].astype(np.float16).T),
            np.ascontiguousarray(v[b].astype(np.float16).T),
        )
        invs[b] = np.ascontiguousarray((~mask[b, 0]).astype(_BF16).T)
    for c in range(N_CORES):
        b, g = c // 2, c % 2
        rows = slice(g * OC, (g + 1) * OC)
        im = {
            "xq_t": xts[b][0],
            "xk_t": xts[b][1],
            "xv_t": xts[b][2],
            "invm_t": invs[b],
            "wq_t": np.ascontiguousarray(wq_w[rows].T).astype(np.float16),
            "wk_t": np.ascontiguousarray(wk_w[rows].T).astype(np.float16),
            "wv_t": np.ascontiguousarray(wv_w[rows].T).astype(np.float16),
            "wo_t": np.ascontiguousarray(wo_w[:, rows].T).astype(_BF16),
        }
        if with_qkv_bias:
            im["bq"] = np.ascontiguousarray(wq_b[rows])
            im["bk"] = np.ascontiguousarray(wk_b[rows])
            im["bv_bcast"] = np.ascontiguousarray(
                np.tile(wv_b[rows][None, :], (P, 1)).astype(np.float32)
            )
        in_maps.append(im)

    return nc, in_maps, with_qkv_bias



# revision 42
# speedup vs baseline: 1.0081x; 1.0081x over previous
"""8-core Trainium2 Bass kernel for nn_Attention_86079734546756.

Sharding: B=4 batches x 2 head-groups (8 heads each) -> 8 cores.
Per core (batch b, head-group g):
  - fp16 projections (Q/K transposed-out, V natural with an appended ones
    column for softmax denominators)
  - scores S^T[k,q] = K_h Q_h^T via fp16 matmuls, two heads packed into the
    128-row PE array (dh=64 contraction each, base_partition 0/64)
  - exp on ScalarE (PSUM->SBUF, bf16 out), mask multiply on VectorE
  - P^T V via bf16 matmuls with M=65 (65th row = ones -> denominators)
  - both heads' denominators staged to SBUF rows 0/64 so ONE batched
    DVE reciprocal (its ~3.3us ucode cost is free-size-bound) serves the
    pair; normalize via reciprocal broadcast across partitions with two
    K=1 bf16 matmuls (hi/lo split for ~fp32 precision)
  - output projection (bf16) -> partial y summed on host (+ wo_b)
Attention emission is software-pipelined (PV of item i-1 after scores of
item i) and a post-pass splits multi-wait instructions because this
walrus build encodes one sync wait per instruction.
No max-subtraction in softmax: |alpha| <~ 60 so exp stays in fp32 range,
and masked lanes are zeroed exactly by multiplying with (1-mask) after exp.
"""

import os
import numpy as np
import ml_dtypes

B, S, D, H = 4, 2048, 1024, 16
DH = D // H          # 64
P = 128
HPC = 8              # heads per core
OC = 512             # output features per core (head-group width)
NKT = S // P         # 16 k-tiles
NQC = S // 512       # 4 q-chunks
NOT = OC // P        # 4 o-tiles
NDT = D // P         # 8 d-tiles
N_CORES = 8

LAST_EXEC_NS = None
LAST_RESULTS = None

_BF16 = ml_dtypes.bfloat16


def _split_multi_waits(nc, mybir, max_waits: int = 1):
    """The walrus build in this container encodes at most one sync wait per
    ISA instruction and refuses to split. Move extra waits onto standalone
    EventSemaphore instructions inserted just before, on the same engine —
    the engine executes them in stream order, so semantics are unchanged
    (DMA triggers are simply enqueued after the waits pass)."""
    ctr = 0
    for fn in nc.m.functions:
        for blk in fn.blocks:
            insts = blk.instructions
            if not any(
                inst.sync_info is not None
                and inst.sync_info.on_wait
                and len(inst.sync_info.on_wait) > max_waits
                for inst in insts
            ):
                continue
            out = []
            for inst in insts:
                si = inst.sync_info
                waits = list(si.on_wait) if si is not None and si.on_wait else []
                if len(waits) > max_waits:
                    extra, keep = waits[:-max_waits], waits[-max_waits:]
                    for w in extra:
                        ev = mybir.InstEventSemaphore(
                            name=f"evsplit-{ctr}",
                            engine=inst.engine,
                            ins=[],
                            outs=[],
                            sync_info=mybir.SyncInfo(on_wait=[w], on_update=[]),
                        )
                        ctr += 1
                        out.append(ev)
                    si.on_wait = keep
                out.append(inst)
            blk.instructions = out
    return ctr


def _build_program(with_qkv_bias: bool):
    from contextlib import ExitStack
    import concourse.bass as bass
    import concourse.mybir as mybir
    import concourse.tile as tile

    dt = mybir.dt
    AF = mybir.ActivationFunctionType
    ALU = mybir.AluOpType

    nc = bass.Bass(trn_type="TRN2")

    xq = nc.declare_dram_parameter("xq_t", [D, S], dt.float16, isOutput=False)
    xk = nc.declare_dram_parameter("xk_t", [D, S], dt.float16, isOutput=False)
    xv = nc.declare_dram_parameter("xv_t", [D, S], dt.float16, isOutput=False)
    invm = nc.declare_dram_parameter("invm_t", [S, S], dt.bfloat16, isOutput=False)
    wq = nc.declare_dram_parameter("wq_t", [D, OC], dt.float16, isOutput=False)
    wk = nc.declare_dram_parameter("wk_t", [D, OC], dt.float16, isOutput=False)
    wv = nc.declare_dram_parameter("wv_t", [D, OC], dt.float16, isOutput=False)
    wo = nc.declare_dram_parameter("wo_t", [OC, D], dt.bfloat16, isOutput=False)
    if with_qkv_bias:
        bq = nc.declare_dram_parameter("bq", [OC], dt.float32, isOutput=False)
        bk = nc.declare_dram_parameter("bk", [OC], dt.float32, isOutput=False)
        bv = nc.declare_dram_parameter("bv_bcast", [P, OC], dt.float32, isOutput=False)
    y = nc.declare_dram_parameter("y_part", [S, D], dt.float32, isOutput=True)

    with tile.TileContext(nc) as tc, ExitStack() as ctx:
        persist = ctx.enter_context(tc.tile_pool(name="persist", bufs=1))
        wpool = ctx.enter_context(tc.tile_pool(name="wpool", bufs=2))
        xpool = ctx.enter_context(tc.tile_pool(name="xpool", bufs=4))
        invp = ctx.enter_context(tc.tile_pool(name="invp", bufs=2))
        ptp = ctx.enter_context(tc.tile_pool(name="ptp", bufs=2))
        yp = ctx.enter_context(tc.tile_pool(name="yp", bufs=3))
        smallp = ctx.enter_context(tc.tile_pool(name="smallp", bufs=1))
        scp = ctx.enter_context(tc.tile_pool(name="scp", bufs=2, space="PSUM"))
        mmp = ctx.enter_context(tc.tile_pool(name="mmp", bufs=3, space="PSUM"))
        bcp = ctx.enter_context(tc.tile_pool(name="bcp", bufs=1, space="PSUM"))

        QHT = persist.tile([P, NOT, S], dt.float16)          # [o%128, ot, s]
        KHT = persist.tile([P, NOT, S], dt.float16)
        VSB = persist.tile([P, NKT, HPC, DH + 1], dt.bfloat16)  # [k%128, kt, h, dh|1]
        OT = persist.tile([P, NOT, S], dt.bfloat16)          # [c%128, ct, s]
        WO = persist.tile([P, NOT, D], dt.bfloat16)          # [c%128, ct, o]
        ones = persist.tile([DH + 1, DH], dt.bfloat16)
        nc.vector.memset(ones[:], 1.0)
        nc.vector.memset(VSB[:, :, :, DH : DH + 1], 1.0)
        # Staging for pair-batched softmax denominators: rows 0 and 64 hold
        # the two heads' denominators (both legal PE base partitions); rows
        # 1-63 stay at 1.0 so the batched reciprocal never sees garbage.
        stages = [
            persist.tile([DH + 1, 512], dt.float32, name=f"stage{i}") for i in range(2)
        ]
        for t in stages:
            nc.vector.memset(t[:], 1.0)
        rcp = persist.tile([DH + 1, 512], dt.float32, name="rcp")

        if with_qkv_bias:
            bq_sb = persist.tile([P, NOT], dt.float32)
            nc.sync.dma_start(bq_sb[:], bq.rearrange("(ot p) -> p ot", p=P))
            bk_sb = persist.tile([P, NOT], dt.float32)
            nc.sync.dma_start(bk_sb[:], bk.rearrange("(ot p) -> p ot", p=P))
            bv_sb = persist.tile([P, OC], dt.float32)
            nc.sync.dma_start(bv_sb[:], bv[:])

        # ---------------- projections -----------------
        # Q/K interleaved per o-tile pair so attention for the first head
        # pairs can start while the rest of the projections still run.
        xqr = xq.rearrange("(dt p) s -> dt p s", p=P)
        xkr = xk.rearrange("(dt p) s -> dt p s", p=P)
        wq_sb = wpool.tile([P, NDT, OC], dt.float16, tag="w", name="wq_sb")
        nc.sync.dma_start(wq_sb[:], wq.rearrange("(dt p) o -> p dt o", p=P))
        wk_sb = wpool.tile([P, NDT, OC], dt.float16, tag="w", name="wk_sb")
        nc.sync.dma_start(wk_sb[:], wk.rearrange("(dt p) o -> p dt o", p=P))

        def proj_full(xr, wsb, dst, bias_sb):
            for sc in range(NQC):
                pss = [
                    scp.tile([P, 2, 512], dt.float32, tag="sc", name=f"pj_{i}")
                    for i in range(2)
                ]
                for dti in range(NDT):
                    xt = xpool.tile([P, 512], dt.float16, tag="x", name="xt")
                    nc.sync.dma_start(xt[:], xr[dti, :, sc * 512 : (sc + 1) * 512])
                    for ot in range(NOT):
                        nc.tensor.matmul(
                            pss[ot // 2][:, ot % 2, :],
                            lhsT=wsb[:, dti, ot * P : (ot + 1) * P],
                            rhs=xt[:],
                            start=(dti == 0),
                            stop=(dti == NDT - 1),
                        )
                for ot in range(NOT):
                    src = pss[ot // 2][:, ot % 2, :]
                    dstap = dst[:, ot, sc * 512 : (sc + 1) * 512]
                    if bias_sb is not None:
                        nc.scalar.activation(
                            dstap, src, AF.Identity, bias=bias_sb[:, ot : ot + 1]
                        )
                    else:
                        nc.scalar.activation(dstap, src, AF.Copy)

        proj_full(xkr, wk_sb, KHT, bk_sb if with_qkv_bias else None)
        proj_full(xqr, wq_sb, QHT, bq_sb if with_qkv_bias else None)

        # V: natural layout [s, o] scattered into VSB[k%128, kt, h, 0:64].
        # Uses the mmp psum tag so attention score tiles don't queue behind it.
        xvr = xv.rearrange("(dt p) s -> dt p s", p=P)
        wvsb = wpool.tile([P, NDT, OC], dt.float16, tag="w", name="wvsb")
        nc.sync.dma_start(wvsb[:], wv.rearrange("(dt p) o -> p dt o", p=P))
        for sc in range(NQC):
            for vh in range(2):
                psa = mmp.tile([P, 512], dt.float32, tag="mm", name=f"pva{sc}_{vh}")
                psb = mmp.tile([P, 512], dt.float32, tag="mm", name=f"pvb{sc}_{vh}")
                for dti in range(NDT):
                    xt = xpool.tile([P, 512], dt.float16, tag="x", name="xtv")
                    nc.sync.dma_start(
                        xt[:], xvr[dti, :, sc * 512 : (sc + 1) * 512]
                    )
                    for i, psx in enumerate((psa, psb)):
                        sti = vh * 2 + i
                        nc.tensor.matmul(
                            psx[:],
                            lhsT=xt[:, sti * P : (sti + 1) * P],
                            rhs=wvsb[:, dti, :],
                            start=(dti == 0),
                            stop=(dti == NDT - 1),
                        )
                for i, psx in enumerate((psa, psb)):
                    st = sc * 4 + vh * 2 + i
                    src = psx[:].rearrange("p (h d) -> p h d", d=DH)
                    dstap = VSB[:, st, :, 0:DH]
                    if with_qkv_bias:
                        nc.vector.tensor_tensor(
                            dstap,
                            src,
                            bv_sb[:].rearrange("p (h d) -> p h d", d=DH),
                            ALU.add,
                        )
                    else:
                        nc.vector.tensor_copy(dstap, src)

        nc.sync.dma_start(WO[:], wo.rearrange("(ct p) o -> p ct o", p=P))

        # ---------------- attention -----------------
        # Software-pipelined emission: PV/normalize for item i-1 are emitted
        # after scores/exp/mask for item i, so the scheduler keeps feeding
        # ScalarE fresh score tiles at iteration boundaries.
        imr = invm.rearrange("(kt p) q -> p kt q", p=P)
        items = [(qc, hp) for qc in range(NQC) for hp in range(NOT)]
        imqs = {}
        pts = {}

        def load_imq(qc):
            if qc in imqs or qc >= NQC:
                return
            qsl = slice(qc * 512, (qc + 1) * 512)
            imq = invp.tile([P, NKT, 512], dt.bfloat16, tag="im", name="imq")
            for k4 in range(4):
                nc.sync.dma_start(
                    imq[:, k4 * 4 : (k4 + 1) * 4, :],
                    imr[:, k4 * 4 : (k4 + 1) * 4, qsl],
                )
            imqs[qc] = imq

        def emit_scores(qc, hp):
            qsl = slice(qc * 512, (qc + 1) * 512)
            load_imq(qc)
            if hp == NOT - 2:
                load_imq(qc + 1)
            imq = imqs[qc]
            PT = ptp.tile([P, NKT, 2, 512], dt.bfloat16, tag="pt", name="PT")
            pts[(qc, hp)] = PT
            for kt in range(NKT):
                ps = scp.tile([P, 2, 512], dt.float32, tag="sc", name="sc")
                ksl = slice(kt * P, (kt + 1) * P)
                nc.tensor.matmul(
                    ps[:, 0, :],
                    lhsT=KHT[0:DH, hp, ksl],
                    rhs=QHT[0:DH, hp, qsl],
                    start=True,
                    stop=True,
                )
                nc.tensor.matmul(
                    ps[:, 1, :],
                    lhsT=KHT[DH:P, hp, ksl],
                    rhs=QHT[DH:P, hp, qsl],
                    start=True,
                    stop=True,
                )
                nc.scalar.activation(PT[:, kt, :, :], ps[:], AF.Exp)
            for j in range(2):
                for q4 in range(4):
                    nc.vector.tensor_tensor(
                        PT[:, q4 * 4 : (q4 + 1) * 4, j, :],
                        PT[:, q4 * 4 : (q4 + 1) * 4, j, :],
                        imq[:, q4 * 4 : (q4 + 1) * 4, :],
                        ALU.mult,
                    )

        stage_idx = [0]

        def emit_pv(qc, hp):
            qsl = slice(qc * 512, (qc + 1) * 512)
            PT = pts.pop((qc, hp))
            si = stage_idx[0]
            stage_idx[0] ^= 1
            stage = stages[si]
            pvs = []
            for j in range(2):
                h = hp * 2 + j
                pv = mmp.tile([P, 512], dt.float32, tag="mm", name="pv")
                for kt in range(NKT):
                    nc.tensor.matmul(
                        pv[0 : DH + 1, :],
                        lhsT=VSB[:, kt, h, :],
                        rhs=PT[:, kt, j, :],
                        start=(kt == 0),
                        stop=(kt == NKT - 1),
                    )
                if j == 0:
                    # Hop through SBUF; the DMA moves the denominator from
                    # partition 64 to row 0 of the staging tile.
                    dtmp = smallp.tile(
                        [DH + 1, 512], dt.float32, tag="dtmp", name="dtmp"
                    )
                    nc.vector.tensor_copy(dtmp[DH : DH + 1, :], pv[DH : DH + 1, :])
                    nc.sync.dma_start(stage[0:1, :], dtmp[DH : DH + 1, :])
                else:
                    nc.vector.tensor_copy(stage[DH : DH + 1, :], pv[DH : DH + 1, :])
                pvs.append(pv)
            # One batched reciprocal serves both heads (rows 0 and 64; rows
            # 1-63 run on the stage's constant 1.0 filler so every lane
            # stays finite); the ~3.3us DVE ucode cost is free-size-bound,
            # not row-bound.
            nc.vector.reciprocal(rcp[:], stage[:])
            # hi/lo bf16 split: bf16 covers the reciprocals' exponent
            # range (down to ~1e-27); accumulating hi+lo in PSUM
            # recovers ~bf16^2 relative precision.
            rc_hi = smallp.tile([DH + 1, 512], dt.bfloat16, tag="rch", name="rc_hi")
            nc.vector.tensor_copy(rc_hi[:], rcp[:])
            nc.vector.tensor_tensor(rcp[:], rcp[:], rc_hi[:], ALU.subtract)
            rc_lo = smallp.tile([DH + 1, 512], dt.bfloat16, tag="rcl", name="rc_lo")
            nc.vector.tensor_copy(rc_lo[:], rcp[:])
            for j in range(2):
                b = j * DH
                bc = bcp.tile([DH, 512], dt.float32, tag="bc", name="bc")
                nc.tensor.matmul(
                    bc[:],
                    lhsT=ones[b : b + 1, :],
                    rhs=rc_hi[b : b + 1, :],
                    start=True,
                    stop=False,
                )
                nc.tensor.matmul(
                    bc[:],
                    lhsT=ones[b : b + 1, :],
                    rhs=rc_lo[b : b + 1, :],
                    start=False,
                    stop=True,
                )
                bcs = smallp.tile([DH, 512], dt.float32, tag="bcs", name="bcs")
                nc.vector.tensor_copy(bcs[:], bc[:])
                nc.vector.tensor_tensor(
                    OT[j * DH : (j + 1) * DH, hp, qsl],
                    pvs[j][0:DH, :],
                    bcs[:],
                    ALU.mult,
                )

        def emit_outproj(qc):
            yr = y.rearrange("(st p) o -> st p o", p=P)
            for sti in range(4):
                st = qc * 4 + sti
                ssl = slice(st * P, (st + 1) * P)
                for oc2 in range(2):
                    osl = slice(oc2 * 512, (oc2 + 1) * 512)
                    op = mmp.tile([P, 512], dt.float32, tag="mm", name="op")
                    for ct in range(NOT):
                        nc.tensor.matmul(
                            op[:],
                            lhsT=OT[:, ct, ssl],
                            rhs=WO[:, ct, osl],
                            start=(ct == 0),
                            stop=(ct == NOT - 1),
                        )
                    yt = yp.tile([P, 512], dt.float32, tag="y", name="yt")
                    nc.vector.tensor_copy(yt[:], op[:])
                    nc.sync.dma_start(yr[st, :, osl], yt[:])

        for idx in range(len(items) + 1):
            if idx < len(items):
                emit_scores(*items[idx])
            if idx > 0:
                pqc, php = items[idx - 1]
                emit_pv(pqc, php)
                if php == NOT - 1:
                    emit_outproj(pqc)

    n_split = _split_multi_waits(nc, mybir)
    return nc


def kernel(q, k, v, mask, wq_w, wq_b, wk_w, wk_b, wv_w, wv_b, wo_w, wo_b):
    global LAST_EXEC_NS, LAST_RESULTS
    from concourse.bass_utils import run_bass_kernel_spmd

    q = np.asarray(q, np.float32)
    k = np.asarray(k, np.float32)
    v = np.asarray(v, np.float32)
    mask = np.asarray(mask)
    wq_w = np.asarray(wq_w, np.float32)
    wk_w = np.asarray(wk_w, np.float32)
    wv_w = np.asarray(wv_w, np.float32)
    wo_w = np.asarray(wo_w, np.float32)
    wq_b = np.asarray(wq_b, np.float32)
    wk_b = np.asarray(wk_b, np.float32)
    wv_b = np.asarray(wv_b, np.float32)
    wo_b = np.asarray(wo_b, np.float32)

    nc, in_maps, with_qkv_bias = prepare(
        q, k, v, mask, wq_w, wq_b, wk_w, wk_b, wv_w, wv_b, wo_w, wo_b
    )

    trace = bool(int(os.environ.get("BASS_ATTN_TRACE", "0")))
    trace_cores = None
    tc_env = os.environ.get("BASS_ATTN_TRACE_CORES", "")
    if tc_env:
        trace_cores = [int(c) for c in tc_env.split(",")]
    res = run_bass_kernel_spmd(
        nc,
        in_maps,
        core_ids=list(range(N_CORES)),
        trace=trace,
        trace_cores=trace_cores,
    )
    LAST_EXEC_NS = res.exec_time_ns
    LAST_RESULTS = res

    out = np.zeros((B, S, D), np.float32)
    for b in range(B):
        out[b] = (
            res.results[2 * b]["y_part"]
            + res.results[2 * b + 1]["y_part"]
            + wo_b[None, :]
        )
    return out


_NC_CACHE = {}


def prepare(q, k, v, mask, wq_w, wq_b, wk_w, wk_b, wv_w, wv_b, wo_w, wo_b):
    with_qkv_bias = bool(
        np.asarray(wq_b).any() or np.asarray(wk_b).any() or np.asarray(wv_b).any()
    )
    if with_qkv_bias not in _NC_CACHE:
        _NC_CACHE[with_qkv_bias] = _build_program(with_qkv_bias)
    nc = _NC_CACHE[with_qkv_bias]

    in_maps = []
    xts = {}
    invs = {}
    for b in range(B):
        # Cast before transposing: halves the bytes the transpose moves.
        xts[b] = (
            np.ascontiguousarray(q[b].astype(np.float16).T),
            np.ascontiguousarray(k[b].astype(np.float16).T),
            np.ascontiguousarray(v[b].astype(np.float16).T),
        )
        invs[b] = np.ascontiguousarray((~mask[b, 0]).astype(_BF16).T)
    for c in range(N_CORES):
        b, g = c // 2, c % 2
        rows = slice(g * OC, (g + 1) * OC)
        im = {
            "xq_t": xts[b][0],
            "xk_t": xts[b][1],
            "xv_t": xts[b][2],
            "invm_t": invs[b],
            "wq_t": np.ascontiguousarray(wq_w[rows].T).astype(np.float16),
            "wk_t": np.ascontiguousarray(wk_w[rows].T).astype(np.float16),
            "wv_t": np.ascontiguousarray(wv_w[rows].T).astype(np.float16),
            "wo_t": np.ascontiguousarray(wo_w[:, rows].T).astype(_BF16),
        }
        if with_qkv_bias:
            im["bq"] = np.ascontiguousarray(wq_b[rows])
            im["bk"] = np.ascontiguousarray(wk_b[rows])
            im["bv_bcast"] = np.ascontiguousarray(
                np.tile(wv_b[rows][None, :], (P, 1)).astype(np.float32)
            )
        in_maps.append(im)

    return nc, in_maps, with_qkv_bias



# revision 44
# speedup vs baseline: 1.1927x; 1.1831x over previous
"""8-core Trainium2 Bass kernel for nn_Attention_86079734546756.

Sharding: B=4 batches x 2 head-groups (8 heads each) -> 8 cores.
Per core (batch b, head-group g):
  - fp16 projections (Q/K transposed-out, V natural with an appended ones
    column for softmax denominators)
  - scores S^T[k,q] = K_h Q_h^T via fp16 matmuls, two heads packed into the
    128-row PE array (dh=64 contraction each, base_partition 0/64)
  - exp on ScalarE (PSUM->SBUF, bf16 out), mask multiply on VectorE
  - P^T V via bf16 matmuls with M=65 (65th row = ones -> denominators)
  - both heads' denominators staged to SBUF rows 0/64 so ONE batched
    DVE reciprocal (its ~3.3us ucode cost is free-size-bound) serves the
    pair; normalize via reciprocal broadcast across partitions with two
    K=1 bf16 matmuls (hi/lo split for ~fp32 precision)
  - output projection (bf16) -> partial y summed on host (+ wo_b)
Attention emission is software-pipelined (PV of item i-1 after scores of
item i) and a post-pass splits multi-wait instructions because this
walrus build encodes one sync wait per instruction.
No max-subtraction in softmax: |alpha| <~ 60 so exp stays in fp32 range,
and masked lanes are zeroed exactly by multiplying with (1-mask) after exp.
"""

import os
import numpy as np
import ml_dtypes

B, S, D, H = 4, 2048, 1024, 16
DH = D // H          # 64
P = 128
HPC = 8              # heads per core
OC = 512             # output features per core (head-group width)
NKT = S // P         # 16 k-tiles
NQC = S // 512       # 4 q-chunks
NOT = OC // P        # 4 o-tiles
NDT = D // P         # 8 d-tiles
N_CORES = 8

LAST_EXEC_NS = None
LAST_RESULTS = None

_BF16 = ml_dtypes.bfloat16


def _split_multi_waits(nc, mybir, max_waits: int = 1):
    """The walrus build in this container encodes at most one sync wait per
    ISA instruction and refuses to split. Move extra waits onto standalone
    EventSemaphore instructions inserted just before, on the same engine —
    the engine executes them in stream order, so semantics are unchanged
    (DMA triggers are simply enqueued after the waits pass)."""
    ctr = 0
    for fn in nc.m.functions:
        for blk in fn.blocks:
            insts = blk.instructions
            if not any(
                inst.sync_info is not None
                and inst.sync_info.on_wait
                and len(inst.sync_info.on_wait) > max_waits
                for inst in insts
            ):
                continue
            out = []
            for inst in insts:
                si = inst.sync_info
                waits = list(si.on_wait) if si is not None and si.on_wait else []
                if len(waits) > max_waits:
                    extra, keep = waits[:-max_waits], waits[-max_waits:]
                    for w in extra:
                        ev = mybir.InstEventSemaphore(
                            name=f"evsplit-{ctr}",
                            engine=inst.engine,
                            ins=[],
                            outs=[],
                            sync_info=mybir.SyncInfo(on_wait=[w], on_update=[]),
                        )
                        ctr += 1
                        out.append(ev)
                    si.on_wait = keep
                out.append(inst)
            blk.instructions = out
    return ctr


def _build_program(with_qkv_bias: bool):
    from contextlib import ExitStack
    import concourse.bass as bass
    import concourse.mybir as mybir
    import concourse.tile as tile

    dt = mybir.dt
    AF = mybir.ActivationFunctionType
    ALU = mybir.AluOpType

    nc = bass.Bass(trn_type="TRN2")

    xq = nc.declare_dram_parameter("xq_t", [D, S], dt.float16, isOutput=False)
    xk = nc.declare_dram_parameter("xk_t", [D, S], dt.float16, isOutput=False)
    xv = nc.declare_dram_parameter("xv_t", [D, S], dt.float16, isOutput=False)
    invm = nc.declare_dram_parameter("invm_t", [S, S], dt.bfloat16, isOutput=False)
    wq = nc.declare_dram_parameter("wq_t", [D, OC], dt.float16, isOutput=False)
    wk = nc.declare_dram_parameter("wk_t", [D, OC], dt.float16, isOutput=False)
    wv = nc.declare_dram_parameter("wv_t", [D, OC], dt.float16, isOutput=False)
    wo = nc.declare_dram_parameter("wo_t", [OC, D], dt.bfloat16, isOutput=False)
    if with_qkv_bias:
        bq = nc.declare_dram_parameter("bq", [OC], dt.float32, isOutput=False)
        bk = nc.declare_dram_parameter("bk", [OC], dt.float32, isOutput=False)
        bv = nc.declare_dram_parameter("bv_bcast", [P, OC], dt.float32, isOutput=False)
    y = nc.declare_dram_parameter("y_part", [S, D], dt.float32, isOutput=True)

    with tile.TileContext(nc) as tc, ExitStack() as ctx:
        persist = ctx.enter_context(tc.tile_pool(name="persist", bufs=1))
        wpool = ctx.enter_context(tc.tile_pool(name="wpool", bufs=2))
        xpool = ctx.enter_context(tc.tile_pool(name="xpool", bufs=4))
        invp = ctx.enter_context(tc.tile_pool(name="invp", bufs=2))
        ptp = ctx.enter_context(tc.tile_pool(name="ptp", bufs=2))
        yp = ctx.enter_context(tc.tile_pool(name="yp", bufs=3))
        smallp = ctx.enter_context(tc.tile_pool(name="smallp", bufs=1))
        scp = ctx.enter_context(tc.tile_pool(name="scp", bufs=2, space="PSUM"))
        mmp = ctx.enter_context(tc.tile_pool(name="mmp", bufs=3, space="PSUM"))
        bcp = ctx.enter_context(tc.tile_pool(name="bcp", bufs=1, space="PSUM"))

        QHT = persist.tile([P, NOT, S], dt.float16)          # [o%128, ot, s]
        KHT = persist.tile([P, NOT, S], dt.float16)
        VSB = persist.tile([P, NKT, HPC, DH + 1], dt.bfloat16)  # [k%128, kt, h, dh|1]
        OT = persist.tile([P, NOT, S], dt.bfloat16)          # [c%128, ct, s]
        WO = persist.tile([P, NOT, D], dt.bfloat16)          # [c%128, ct, o]
        ones32 = persist.tile([DH + 1, DH], dt.float32)
        nc.vector.memset(ones32[:], 1.0)
        nc.vector.memset(VSB[:, :, :, DH : DH + 1], 1.0)
        # Staging for pair-batched softmax denominators: rows 0 and 64 hold
        # the two heads' denominators (both legal PE base partitions); rows
        # 1-63 stay at 1.0 so the batched reciprocal never sees garbage.
        stages = [
            persist.tile([DH + 1, 512], dt.float32, name=f"stage{i}") for i in range(2)
        ]
        for t in stages:
            nc.vector.memset(t[:], 1.0)
        rcp = persist.tile([DH + 1, 512], dt.float32, name="rcp")

        if with_qkv_bias:
            bq_sb = persist.tile([P, NOT], dt.float32)
            nc.sync.dma_start(bq_sb[:], bq.rearrange("(ot p) -> p ot", p=P))
            bk_sb = persist.tile([P, NOT], dt.float32)
            nc.sync.dma_start(bk_sb[:], bk.rearrange("(ot p) -> p ot", p=P))
            bv_sb = persist.tile([P, OC], dt.float32)
            nc.sync.dma_start(bv_sb[:], bv[:])

        # ---------------- projections -----------------
        # Q/K interleaved per o-tile pair so attention for the first head
        # pairs can start while the rest of the projections still run.
        xqr = xq.rearrange("(dt p) s -> dt p s", p=P)
        xkr = xk.rearrange("(dt p) s -> dt p s", p=P)
        wq_sb = wpool.tile([P, NDT, OC], dt.float16, tag="w", name="wq_sb")
        wk_sb = wpool.tile([P, NDT, OC], dt.float16, tag="w", name="wk_sb")
        wqrr = wq.rearrange("(dt p) o -> p dt o", p=P)
        wkrr = wk.rearrange("(dt p) o -> p dt o", p=P)
        for dti in range(NDT):
            nc.sync.dma_start(wk_sb[:, dti, :], wkrr[:, dti, :])
        for dti in range(NDT):
            nc.sync.dma_start(wq_sb[:, dti, :], wqrr[:, dti, :])

        def proj_full(xr, wsb, dst, bias_sb):
            for sc in range(NQC):
                pss = [
                    scp.tile([P, 2, 512], dt.float32, tag="sc", name=f"pj_{i}")
                    for i in range(2)
                ]
                for dti in range(NDT):
                    xt = xpool.tile([P, 512], dt.float16, tag="x", name="xt")
                    nc.sync.dma_start(xt[:], xr[dti, :, sc * 512 : (sc + 1) * 512])
                    for ot in range(NOT):
                        nc.tensor.matmul(
                            pss[ot // 2][:, ot % 2, :],
                            lhsT=wsb[:, dti, ot * P : (ot + 1) * P],
                            rhs=xt[:],
                            start=(dti == 0),
                            stop=(dti == NDT - 1),
                        )
                for ot in range(NOT):
                    src = pss[ot // 2][:, ot % 2, :]
                    dstap = dst[:, ot, sc * 512 : (sc + 1) * 512]
                    if bias_sb is not None:
                        nc.scalar.activation(
                            dstap, src, AF.Identity, bias=bias_sb[:, ot : ot + 1]
                        )
                    elif ot % 2 == 0:
                        nc.scalar.activation(dstap, src, AF.Copy)
                    else:
                        nc.vector.tensor_copy(dstap, src)

        proj_full(xkr, wk_sb, KHT, bk_sb if with_qkv_bias else None)
        proj_full(xqr, wq_sb, QHT, bq_sb if with_qkv_bias else None)

        # V: natural layout [s, o] scattered into VSB[k%128, kt, h, 0:64].
        # Uses the mmp psum tag so attention score tiles don't queue behind it.
        xvr = xv.rearrange("(dt p) s -> dt p s", p=P)
        wvsb = wpool.tile([P, NDT, OC], dt.float16, tag="w", name="wvsb")
        nc.sync.dma_start(wvsb[:], wv.rearrange("(dt p) o -> p dt o", p=P))
        for sc in range(NQC):
            for vh in range(2):
                psa = mmp.tile([P, 512], dt.float32, tag="mm", name=f"pva{sc}_{vh}")
                psb = mmp.tile([P, 512], dt.float32, tag="mm", name=f"pvb{sc}_{vh}")
                for dti in range(NDT):
                    xt = xpool.tile([P, 512], dt.float16, tag="x", name="xtv")
                    nc.sync.dma_start(
                        xt[:], xvr[dti, :, sc * 512 : (sc + 1) * 512]
                    )
                    for i, psx in enumerate((psa, psb)):
                        sti = vh * 2 + i
                        nc.tensor.matmul(
                            psx[:],
                            lhsT=xt[:, sti * P : (sti + 1) * P],
                            rhs=wvsb[:, dti, :],
                            start=(dti == 0),
                            stop=(dti == NDT - 1),
                        )
                for i, psx in enumerate((psa, psb)):
                    st = sc * 4 + vh * 2 + i
                    src = psx[:].rearrange("p (h d) -> p h d", d=DH)
                    dstap = VSB[:, st, :, 0:DH]
                    if with_qkv_bias:
                        nc.vector.tensor_tensor(
                            dstap,
                            src,
                            bv_sb[:].rearrange("p (h d) -> p h d", d=DH),
                            ALU.add,
                        )
                    else:
                        nc.vector.tensor_copy(dstap, src)

        nc.sync.dma_start(WO[:], wo.rearrange("(ct p) o -> p ct o", p=P))

        # ---------------- attention -----------------
        # Software-pipelined emission: PV/normalize for item i-1 are emitted
        # after scores/exp/mask for item i, so the scheduler keeps feeding
        # ScalarE fresh score tiles at iteration boundaries.
        imr = invm.rearrange("(kt p) q -> p kt q", p=P)
        items = [(qc, hp) for qc in range(NQC) for hp in range(NOT)]
        imqs = {}
        pts = {}

        def load_imq(qc):
            if qc in imqs or qc >= NQC:
                return
            qsl = slice(qc * 512, (qc + 1) * 512)
            imq = invp.tile([P, NKT, 512], dt.bfloat16, tag="im", name="imq")
            for k4 in range(4):
                nc.sync.dma_start(
                    imq[:, k4 * 4 : (k4 + 1) * 4, :],
                    imr[:, k4 * 4 : (k4 + 1) * 4, qsl],
                )
            imqs[qc] = imq

        def emit_scores(qc, hp):
            qsl = slice(qc * 512, (qc + 1) * 512)
            load_imq(qc)
            if hp == NOT - 2:
                load_imq(qc + 1)
            imq = imqs[qc]
            PT = ptp.tile([P, NKT, 2, 512], dt.bfloat16, tag="pt", name="PT")
            pts[(qc, hp)] = PT
            for kt in range(NKT):
                ps = scp.tile([P, 2, 512], dt.float32, tag="sc", name="sc")
                ksl = slice(kt * P, (kt + 1) * P)
                nc.tensor.matmul(
                    ps[:, 0, :],
                    lhsT=KHT[0:DH, hp, ksl],
                    rhs=QHT[0:DH, hp, qsl],
                    start=True,
                    stop=True,
                )
                nc.tensor.matmul(
                    ps[:, 1, :],
                    lhsT=KHT[DH:P, hp, ksl],
                    rhs=QHT[DH:P, hp, qsl],
                    start=True,
                    stop=True,
                )
                nc.scalar.activation(PT[:, kt, :, :], ps[:], AF.Exp)
            for j in range(2):
                for q4 in range(4):
                    nc.vector.tensor_tensor(
                        PT[:, q4 * 4 : (q4 + 1) * 4, j, :],
                        PT[:, q4 * 4 : (q4 + 1) * 4, j, :],
                        imq[:, q4 * 4 : (q4 + 1) * 4, :],
                        ALU.mult,
                    )

        stage_idx = [0]

        def emit_pv(qc, hp):
            qsl = slice(qc * 512, (qc + 1) * 512)
            PT = pts.pop((qc, hp))
            si = stage_idx[0]
            stage_idx[0] ^= 1
            stage = stages[si]
            pvs = []
            for j in range(2):
                h = hp * 2 + j
                pv = mmp.tile([P, 512], dt.float32, tag="mm", name="pv")
                for kt in range(NKT):
                    nc.tensor.matmul(
                        pv[0 : DH + 1, :],
                        lhsT=VSB[:, kt, h, :],
                        rhs=PT[:, kt, j, :],
                        start=(kt == 0),
                        stop=(kt == NKT - 1),
                    )
                if j == 0:
                    # Hop through SBUF; the DMA moves the denominator from
                    # partition 64 to row 0 of the staging tile.
                    dtmp = smallp.tile(
                        [DH + 1, 512], dt.float32, tag="dtmp", name="dtmp"
                    )
                    nc.vector.tensor_copy(dtmp[DH : DH + 1, :], pv[DH : DH + 1, :])
                    nc.sync.dma_start(stage[0:1, :], dtmp[DH : DH + 1, :])
                else:
                    nc.vector.tensor_copy(stage[DH : DH + 1, :], pv[DH : DH + 1, :])
                pvs.append(pv)
            # One batched reciprocal serves both heads (rows 0 and 64; rows
            # 1-63 run on the stage's constant 1.0 filler so every lane
            # stays finite); the ~3.3us DVE ucode cost is free-size-bound,
            # not row-bound.
            nc.vector.reciprocal(rcp[:], stage[:])
            for j in range(2):
                b = j * DH
                bc = bcp.tile([DH, 512], dt.float32, tag="bc", name="bc")
                # Single fp32 K=1 matmul broadcasts the full-precision
                # reciprocal; the bf16 hi/lo cast/subtract tail disappears
                # from the chain the PE waits on.
                nc.tensor.matmul(
                    bc[:],
                    lhsT=ones32[b : b + 1, :],
                    rhs=rcp[b : b + 1, :],
                    start=True,
                    stop=True,
                )
                bcs = smallp.tile([DH, 512], dt.float32, tag="bcs", name="bcs")
                nc.vector.tensor_copy(bcs[:], bc[:])
                nc.vector.tensor_tensor(
                    OT[j * DH : (j + 1) * DH, hp, qsl],
                    pvs[j][0:DH, :],
                    bcs[:],
                    ALU.mult,
                )

        def emit_outproj(qc):
            yr = y.rearrange("(st p) o -> st p o", p=P)
            for sti in range(4):
                st = qc * 4 + sti
                ssl = slice(st * P, (st + 1) * P)
                for oc2 in range(2):
                    osl = slice(oc2 * 512, (oc2 + 1) * 512)
                    op = mmp.tile([P, 512], dt.float32, tag="mm", name="op")
                    for ct in range(NOT):
                        nc.tensor.matmul(
                            op[:],
                            lhsT=OT[:, ct, ssl],
                            rhs=WO[:, ct, osl],
                            start=(ct == 0),
                            stop=(ct == NOT - 1),
                        )
                    yt = yp.tile([P, 512], dt.float32, tag="y", name="yt")
                    nc.vector.tensor_copy(yt[:], op[:])
                    nc.sync.dma_start(yr[st, :, osl], yt[:])

        for idx in range(len(items) + 1):
            if idx < len(items):
                emit_scores(*items[idx])
            if idx > 0:
                pqc, php = items[idx - 1]
                emit_pv(pqc, php)
                if php == NOT - 1:
                    emit_outproj(pqc)

    n_split = _split_multi_waits(nc, mybir)
    return nc


def kernel(q, k, v, mask, wq_w, wq_b, wk_w, wk_b, wv_w, wv_b, wo_w, wo_b):
    global LAST_EXEC_NS, LAST_RESULTS
    from concourse.bass_utils import run_bass_kernel_spmd

    q = np.asarray(q, np.float32)
    k = np.asarray(k, np.float32)
    v = np.asarray(v, np.float32)
    mask = np.asarray(mask)
    wq_w = np.asarray(wq_w, np.float32)
    wk_w = np.asarray(wk_w, np.float32)
    wv_w = np.asarray(wv_w, np.float32)
    wo_w = np.asarray(wo_w, np.float32)
    wq_b = np.asarray(wq_b, np.float32)
    wk_b = np.asarray(wk_b, np.float32)
    wv_b = np.asarray(wv_b, np.float32)
    wo_b = np.asarray(wo_b, np.float32)

    nc, in_maps, with_qkv_bias = prepare(
        q, k, v, mask, wq_w, wq_b, wk_w, wk_b, wv_w, wv_b, wo_w, wo_b
    )

    trace = bool(int(os.environ.get("BASS_ATTN_TRACE", "0")))
    trace_cores = None
    tc_env = os.environ.get("BASS_ATTN_TRACE_CORES", "")
    if tc_env:
        trace_cores = [int(c) for c in tc_env.split(",")]
    res = run_bass_kernel_spmd(
        nc,
        in_maps,
        core_ids=list(range(N_CORES)),
        trace=trace,
        trace_cores=trace_cores,
    )
    LAST_EXEC_NS = res.exec_time_ns
    LAST_RESULTS = res

    out = np.zeros((B, S, D), np.float32)
    for b in range(B):
        out[b] = (
            res.results[2 * b]["y_part"]
            + res.results[2 * b + 1]["y_part"]
            + wo_b[None, :]
        )
    return out


_NC_CACHE = {}


def prepare(q, k, v, mask, wq_w, wq_b, wk_w, wk_b, wv_w, wv_b, wo_w, wo_b):
    with_qkv_bias = bool(
        np.asarray(wq_b).any() or np.asarray(wk_b).any() or np.asarray(wv_b).any()
    )
    if with_qkv_bias not in _NC_CACHE:
        _NC_CACHE[with_qkv_bias] = _build_program(with_qkv_bias)
    nc = _NC_CACHE[with_qkv_bias]

    in_maps = []
    xts = {}
    invs = {}
    for b in range(B):
        # Cast before transposing: halves the bytes the transpose moves.
        xts[b] = (
            np.ascontiguousarray(q[b].astype(np.float16).T),
            np.ascontiguousarray(k[b].astype(np.float16).T),
            np.ascontiguousarray(v[b].astype(np.float16).T),
        )
        invs[b] = np.ascontiguousarray((~mask[b, 0]).astype(_BF16).T)
    for c in range(N_CORES):
        b, g = c // 2, c % 2
        rows = slice(g * OC, (g + 1) * OC)
        im = {
            "xq_t": xts[b][0],
            "xk_t": xts[b][1],
            "xv_t": xts[b][2],
            "invm_t": invs[b],
            "wq_t": np.ascontiguousarray(wq_w[rows].T).astype(np.float16),
            "wk_t": np.ascontiguousarray(wk_w[rows].T).astype(np.float16),
            "wv_t": np.ascontiguousarray(wv_w[rows].T).astype(np.float16),
            "wo_t": np.ascontiguousarray(wo_w[:, rows].T).astype(_BF16),
        }
        if with_qkv_bias:
            im["bq"] = np.ascontiguousarray(wq_b[rows])
            im["bk"] = np.ascontiguousarray(wk_b[rows])
            im["bv_bcast"] = np.ascontiguousarray(
                np.tile(wv_b[rows][None, :], (P, 1)).astype(np.float32)
            )
        in_maps.append(im)

    return nc, in_maps, with_qkv_bias

